# revision 17
# baseline (speedup 1.0000x reference)
"""Trainium2 Bass kernel for nn_Loss5 (topk_masking).

reference:
    s_topk = top_k(x, 6)[0][:, 5]            # 6th largest per row, [B]
    s_y    = x[arange(B), y]                 # label score, [B]
    out    = mean(relu(1 + s_topk[None,:] - s_y[:,None]))   # over [B,B]

Key structural fact: for this problem's input distribution (x ~ N(0,1),
V = 50257 columns) the hinge is never active -- s_topk is the 6th
largest of 50257 normals (>= ~3.3 for every row) while s_y is a single
normal draw (<= ~3.5 max over 4096 rows), so 1 + s_topk - s_y >= ~1
for every pair.  With no pair clipped,

    mean(relu(1 + t_j - s_i)) == 1 + mean(t) - mean(s)

exactly.  The kernel therefore computes per-row s_topk (exact top-6 via
DVE Max8 over the full 50257 columns -- the memory-bound part is
untouched) and the gathered s_y, and ships both [512]-vectors per core
back to the host, which applies the linear formula (and VERIFIES the
no-clip margin; if the margin were ever violated it falls back to the
exact O(B^2) hinge evaluated from the same t/s vectors, so the kernel
is correct for any input).

This removes all cross-core communication (the ncfw AllGather measured
~300 us per call on this stack -- dwarfing the 260 us compute) and the
whole [B,B] accumulation stage.

Stage 1 (per core, 512 rows of x as 4 groups x 128 partitions):
  - column chunks sized <= 16384 (Max8 input limit), loaded with an
    f32 -> bf16 cast during DMA (SWDGE/gpsimd path; halves SBUF-write
    traffic, measured a few percent faster than f32 HWDGE loads at the
    same ~410 GB/s/core read roofline) into 6 rotating SBUF slots.
  - per chunk: DVE Max8 -> 8 candidates; per group: final Max8 over
    the chunk candidates; s_topk = 6th value; copied (bf16 -> f32)
    into the output tile.  The last chunk is deliberately small so the
    end-of-pipe (last Max8 after the last DMA) is short.  bf16
    rounding of s_topk/s_y costs rel err ~1.4e-05 (gate is 2e-2).
  - s_y via indirect DMA gather (host-computed flat offsets) on gpsimd.

Raw bass blocks (not Tile): DMA pseudo-instructions support only ONE
attached sync wait; with explicit semaphores every DMA carries at most
one wait (WAW implied transitively).
"""

import sys

import numpy as np

if "/opt/trn_rl_repo" not in sys.path:
    sys.path.insert(0, "/opt/trn_rl_repo")

import concourse.bass as bass
import concourse.mybir as mybir
from concourse.bass_utils import run_bass_kernel_spmd

B = 4096
V = 50257
NCORES = 8
RPC = B // NCORES          # rows per core = 512
G = RPC // 128             # row groups of 128 partitions = 4
K = 5                      # s_topk = (K+1)-th largest = top8[:, 5]

# column chunk widths for Max8 (input free size must be <= 16384).
# Large chunks for DMA efficiency; small last chunk so the final
# Max8 after the last load is short.
CHUNKS_DEFAULT = (12000, 12000, 12000, 12000, 2257)
assert sum(CHUNKS_DEFAULT) == V

NSLOT_DEFAULT = 6          # x-tile load slots (bf16 slots are cheap)

_NC_CACHE = {}


def _build_nc(
    repeat: int = 1,
    dve_min: bool = False,
    chunks: tuple = CHUNKS_DEFAULT,
    nslot: int = NSLOT_DEFAULT,
    two_queues: bool = False,
    cast16: bool = True,
    no_drain: bool = False,
    k_idx: int = K,
):
    f32 = mybir.dt.float32
    i32 = mybir.dt.int32
    xdt = mybir.dt.bfloat16 if cast16 else f32

    nchunk = len(chunks)
    w0 = max(chunks)
    starts = []
    c0 = 0
    for w in chunks:
        starts.append(c0)
        c0 += w
    assert c0 == V

    nc = bass.Bass()
    x = nc.declare_dram_parameter("x", [RPC, V], f32, isOutput=False)
    syoff = nc.declare_dram_parameter("syoff", [128, G], i32, isOutput=False)
    # output: cols 0:G = s_topk per group, G:2G = s_y per group
    osb_d = nc.declare_dram_parameter("osb", [128, 2 * G], f32, isOutput=True)

    x_flat = x.ap().rearrange("a b -> (a b)")[:, None]

    from contextlib import ExitStack

    with ExitStack() as ctx:
        slots = ctx.enter_context(nc.sbuf_tensor("slots", [128, nslot * w0], xdt))
        cand = ctx.enter_context(nc.sbuf_tensor("cand", [128, G * 8 * nchunk], xdt))
        top8 = ctx.enter_context(nc.sbuf_tensor("top8", [128, G * 8], xdt))
        osb = ctx.enter_context(nc.sbuf_tensor("osbt", [128, 2 * G], f32))
        so_sb = ctx.enter_context(nc.sbuf_tensor("so", [128, G], i32))
        ld_sems = [
            ctx.enter_context(nc.semaphore(f"ld{i}")) for i in range(nslot)
        ]
        mx = ctx.enter_context(nc.semaphore("mx"))
        fmx = ctx.enter_context(nc.semaphore("fmx"))
        tcp = ctx.enter_context(nc.semaphore("tcp"))
        so_s = ctx.enter_context(nc.semaphore("so_s"))
        gat = ctx.enter_context(nc.semaphore("gat"))
        outs = ctx.enter_context(nc.semaphore("outs"))
        block = ctx.enter_context(nc.Block(no_gpsimd_drain=no_drain))

        def emit_loads(q, parity):
            # parity None -> all loads on this queue; else only k%2==parity
            k = 0
            for rep in range(repeat):
                for g in range(G):
                    for j in range(nchunk):
                        if parity is None or (k % 2) == parity:
                            if k >= nslot:
                                q.wait_ge(mx, k - nslot + 1)
                            s = (k % nslot) * w0
                            w = chunks[j]
                            c0 = starts[j]
                            cs = (c0 + rep * 1237) % (V - w) if rep else c0
                            q.dma_start(
                                out=slots[:, s : s + w],
                                in_=x[g * 128 : (g + 1) * 128, cs : cs + w],
                            ).then_inc(ld_sems[k % nslot], 16)
                        k += 1

        @block.sync
        def _(sync):
            if not cast16:
                emit_loads(sync, 0 if two_queues else None)
            if repeat > 0:
                sync.wait_ge(tcp, G * repeat)
                sync.wait_ge(gat, 16 * G)
                sync.dma_start(out=osb_d.ap(), in_=osb[:]).then_inc(outs, 16)
                sync.wait_ge(outs, 16)

        if two_queues:

            @block.scalar
            def _(scalar):
                emit_loads(scalar, 1)

        @block.vector
        def _(vector):
            k = 0
            nc8 = 8 * nchunk
            for rep in range(repeat):
                for g in range(G):
                    for j in range(nchunk):
                        s = (k % nslot) * w0
                        w = chunks[j]
                        vector.wait_ge(ld_sems[k % nslot], 16 * (k // nslot + 1))
                        nc.vector.max(
                            cand[:, nc8 * g + 8 * j : nc8 * g + 8 * j + 8],
                            slots[:, s : s + (8 if dve_min else w)],
                        ).then_inc(mx, 1)
                        k += 1
                    vector.wait_ge(mx, nchunk * (rep * G + g + 1))
                    nc.vector.max(
                        top8[:, 8 * g : 8 * g + 8], cand[:, nc8 * g : nc8 * (g + 1)]
                    ).then_inc(fmx, 1)
                    vector.wait_ge(fmx, rep * G + g + 1)
                    nc.vector.tensor_copy(
                        osb[:, g : g + 1],
                        top8[:, 8 * g + k_idx : 8 * g + k_idx + 1],
                    ).then_inc(tcp, 1)

        @block.gpsimd
        def _(gpsimd):
            if repeat > 0:
                gpsimd.dma_start(out=so_sb[:], in_=syoff.ap()).then_inc(so_s, 16)
                gpsimd.wait_ge(so_s, 16)
                for g in range(G):
                    gpsimd.indirect_dma_start(
                        out=osb[:, G + g : G + g + 1],
                        out_offset=None,
                        in_=x_flat,
                        in_offset=bass.IndirectOffsetOnAxis(
                            ap=so_sb[:, g : g + 1], axis=0
                        ),
                    ).then_inc(gat, 16)
            if cast16:
                # f32 -> bf16 cast during DMA is SWDGE-only
                emit_loads(gpsimd, None)

    return nc


def _get_nc(repeat: int = 1, **kw):
    key = ("nc", repeat, tuple(sorted(kw.items())))
    if key not in _NC_CACHE:
        _NC_CACHE[key] = _build_nc(repeat, **kw)
    return _NC_CACHE[key]


def _make_in_maps(x, y):
    in_maps = []
    r = np.arange(RPC, dtype=np.int64)
    for c in range(NCORES):
        rows = slice(c * RPC, (c + 1) * RPC)
        yl = y[rows]
        off = (r * V + yl).astype(np.int32).reshape(G, 128).T.copy()
        in_maps.append({"x": x[rows], "syoff": off})
    return in_maps


def _combine(per_core_osb):
    """per_core_osb: list of [128, 2G] arrays -> final scalar (np.float32).

    t/s layout: osb[p, g] is local row g*128+p.  Applies the linear
    formula after verifying the no-clip margin; exact hinge fallback
    otherwise (never taken for this problem's input distribution).
    """
    t = np.empty(B, dtype=np.float64)
    s = np.empty(B, dtype=np.float64)
    for c, o in enumerate(per_core_osb):
        o = np.asarray(o, dtype=np.float64)
        for g in range(G):
            rows = slice(c * RPC + g * 128, c * RPC + (g + 1) * 128)
            t[rows] = o[:, g]
            s[rows] = o[:, G + g]
    if 1.0 + t.min() - s.max() >= 0.0:
        out = 1.0 + t.mean() - s.mean()
    else:
        out = np.mean(np.maximum(1.0 + t[None, :] - s[:, None], 0.0))
    return np.array(out, dtype=np.float32)


def _run(x, y, k_idx=K, trace=False):
    x = np.ascontiguousarray(np.asarray(x, dtype=np.float32))
    y = np.asarray(y).astype(np.int64).reshape(B)
    assert x.shape == (B, V)

    nc = _get_nc(k_idx=k_idx)
    in_maps = _make_in_maps(x, y)
    res = run_bass_kernel_spmd(nc, in_maps, list(range(NCORES)), trace=trace)
    out = _combine([res.results[c]["osb"] for c in range(NCORES)])
    return out, res


def kernel(x, y, k):
    k = int(k)
    assert 0 <= k <= 7, f"top-8 selection supports k+1 <= 8, got k={k}"
    out, _ = _run(x, y, k_idx=k, trace=False)
    return out


# revision 20
# speedup vs baseline: 1.1291x; 1.1291x over previous
"""Trainium2 Bass kernel for nn_Loss5 (topk_masking).

reference:
    s_topk = top_k(x, 6)[0][:, 5]            # 6th largest per row, [B]
    s_y    = x[arange(B), y]                 # label score, [B]
    out    = mean(relu(1 + s_topk[None,:] - s_y[:,None]))   # over [B,B]

Key structural fact: for this problem's input distribution (x ~ N(0,1),
V = 50257 columns) the hinge is never active -- s_topk is the 6th
largest of 50257 normals (>= ~3.3 for every row) while s_y is a single
normal draw (<= ~3.5 max over 4096 rows), so 1 + s_topk - s_y >= ~1
for every pair.  With no pair clipped,

    mean(relu(1 + t_j - s_i)) == 1 + mean(t) - mean(s)

exactly.  The kernel therefore computes per-row s_topk (exact top-6 via
DVE Max8 over the full 50257 columns -- the memory-bound part is
untouched) and the gathered s_y, and ships both [512]-vectors per core
back to the host, which applies the linear formula (and VERIFIES the
no-clip margin; if the margin were ever violated it falls back to the
exact O(B^2) hinge evaluated from the same t/s vectors, so the kernel
is correct for any input).

This removes all cross-core communication (the ncfw AllGather measured
~300 us per call on this stack -- dwarfing the 260 us compute) and the
whole [B,B] accumulation stage.

Stage 1 (per core, 512 rows of x as 4 groups x 128 partitions):
  - column chunks sized <= 16384 (Max8 input limit), f32 HWDGE loads
    on the sync queue into 3 rotating SBUF slots (measured ~400-413
    GB/s/core -- the read-side roofline; DVE hides fully under DMA).
    A cast16 variant (f32 -> bf16 SWDGE cast during DMA) measured a
    few percent faster but wedged the device sporadically
    (NRT_EXEC_UNIT_UNRECOVERABLE, plausibly the inline-compute 2048-
    element descriptor limit); f32/HWDGE never wedged, so it is the
    default.
  - per chunk: DVE Max8 -> 8 candidates; per group: final Max8 over
    the chunk candidates; s_topk = 6th value; copied into the output
    tile.  The last chunk is deliberately small so the end-of-pipe
    (last Max8 after the last DMA) is short.
  - s_y via indirect DMA gather (host-computed flat offsets) on gpsimd.

Raw bass blocks (not Tile): DMA pseudo-instructions support only ONE
attached sync wait; with explicit semaphores every DMA carries at most
one wait (WAW implied transitively).
"""

import sys

import numpy as np

if "/opt/trn_rl_repo" not in sys.path:
    sys.path.insert(0, "/opt/trn_rl_repo")

import concourse.bass as bass
import concourse.mybir as mybir
from concourse.bass_utils import run_bass_kernel_spmd

B = 4096
V = 50257
NCORES = 8
RPC = B // NCORES          # rows per core = 512
G = RPC // 128             # row groups of 128 partitions = 4
K = 5                      # s_topk = (K+1)-th largest = top8[:, 5]

# column chunk widths for Max8 (input free size must be <= 16384).
# Large chunks for DMA efficiency; small last chunk so the final
# Max8 after the last load is short.
CHUNKS_DEFAULT = (12000, 12000, 12000, 12000, 2257)
assert sum(CHUNKS_DEFAULT) == V

NSLOT_DEFAULT = 3          # x-tile load slots (triple buffering)

_NC_CACHE = {}


def _build_nc(
    repeat: int = 1,
    dve_min: bool = False,
    chunks: tuple = CHUNKS_DEFAULT,
    nslot: int = NSLOT_DEFAULT,
    two_queues: bool = False,
    cast16: bool = False,
    no_drain: bool = False,
    k_idx: int = K,
):
    f32 = mybir.dt.float32
    i32 = mybir.dt.int32
    xdt = mybir.dt.bfloat16 if cast16 else f32

    nchunk = len(chunks)
    w0 = max(chunks)
    starts = []
    c0 = 0
    for w in chunks:
        starts.append(c0)
        c0 += w
    assert c0 == V

    nc = bass.Bass()
    x = nc.declare_dram_parameter("x", [RPC, V], f32, isOutput=False)
    syoff = nc.declare_dram_parameter("syoff", [128, G], i32, isOutput=False)
    # output: cols 0:G = s_topk per group, G:2G = s_y per group
    osb_d = nc.declare_dram_parameter("osb", [128, 2 * G], f32, isOutput=True)

    x_flat = x.ap().rearrange("a b -> (a b)")[:, None]

    from contextlib import ExitStack

    with ExitStack() as ctx:
        slots = ctx.enter_context(nc.sbuf_tensor("slots", [128, nslot * w0], xdt))
        cand = ctx.enter_context(nc.sbuf_tensor("cand", [128, G * 8 * nchunk], xdt))
        top8 = ctx.enter_context(nc.sbuf_tensor("top8", [128, G * 8], xdt))
        osb = ctx.enter_context(nc.sbuf_tensor("osbt", [128, 2 * G], f32))
        so_sb = ctx.enter_context(nc.sbuf_tensor("so", [128, G], i32))
        ld_sems = [
            ctx.enter_context(nc.semaphore(f"ld{i}")) for i in range(nslot)
        ]
        mx = ctx.enter_context(nc.semaphore("mx"))
        fmx = ctx.enter_context(nc.semaphore("fmx"))
        tcp = ctx.enter_context(nc.semaphore("tcp"))
        so_s = ctx.enter_context(nc.semaphore("so_s"))
        gat = ctx.enter_context(nc.semaphore("gat"))
        outs = ctx.enter_context(nc.semaphore("outs"))
        block = ctx.enter_context(nc.Block(no_gpsimd_drain=no_drain))

        def emit_loads(q, parity):
            # parity None -> all loads on this queue; else only k%2==parity
            k = 0
            for rep in range(repeat):
                for g in range(G):
                    for j in range(nchunk):
                        if parity is None or (k % 2) == parity:
                            if k >= nslot:
                                q.wait_ge(mx, k - nslot + 1)
                            s = (k % nslot) * w0
                            w = chunks[j]
                            c0 = starts[j]
                            cs = (c0 + rep * 1237) % (V - w) if rep else c0
                            q.dma_start(
                                out=slots[:, s : s + w],
                                in_=x[g * 128 : (g + 1) * 128, cs : cs + w],
                            ).then_inc(ld_sems[k % nslot], 16)
                        k += 1

        @block.sync
        def _(sync):
            if not cast16:
                emit_loads(sync, 0 if two_queues else None)
            if repeat > 0:
                sync.wait_ge(tcp, G * repeat)
                sync.wait_ge(gat, 16 * G)
                sync.dma_start(out=osb_d.ap(), in_=osb[:]).then_inc(outs, 16)
                sync.wait_ge(outs, 16)

        if two_queues:

            @block.scalar
            def _(scalar):
                emit_loads(scalar, 1)

        @block.vector
        def _(vector):
            k = 0
            nc8 = 8 * nchunk
            for rep in range(repeat):
                for g in range(G):
                    for j in range(nchunk):
                        s = (k % nslot) * w0
                        w = chunks[j]
                        vector.wait_ge(ld_sems[k % nslot], 16 * (k // nslot + 1))
                        nc.vector.max(
                            cand[:, nc8 * g + 8 * j : nc8 * g + 8 * j + 8],
                            slots[:, s : s + (8 if dve_min else w)],
                        ).then_inc(mx, 1)
                        k += 1
                    vector.wait_ge(mx, nchunk * (rep * G + g + 1))
                    nc.vector.max(
                        top8[:, 8 * g : 8 * g + 8], cand[:, nc8 * g : nc8 * (g + 1)]
                    ).then_inc(fmx, 1)
                    vector.wait_ge(fmx, rep * G + g + 1)
                    nc.vector.tensor_copy(
                        osb[:, g : g + 1],
                        top8[:, 8 * g + k_idx : 8 * g + k_idx + 1],
                    ).then_inc(tcp, 1)

        @block.gpsimd
        def _(gpsimd):
            if repeat > 0:
                gpsimd.dma_start(out=so_sb[:], in_=syoff.ap()).then_inc(so_s, 16)
                gpsimd.wait_ge(so_s, 16)
                for g in range(G):
                    gpsimd.indirect_dma_start(
                        out=osb[:, G + g : G + g + 1],
                        out_offset=None,
                        in_=x_flat,
                        in_offset=bass.IndirectOffsetOnAxis(
                            ap=so_sb[:, g : g + 1], axis=0
                        ),
                    ).then_inc(gat, 16)
            if cast16:
                # f32 -> bf16 cast during DMA is SWDGE-only
                emit_loads(gpsimd, None)

    return nc


def _get_nc(repeat: int = 1, **kw):
    key = ("nc", repeat, tuple(sorted(kw.items())))
    if key not in _NC_CACHE:
        _NC_CACHE[key] = _build_nc(repeat, **kw)
    return _NC_CACHE[key]


def _make_in_maps(x, y):
    in_maps = []
    r = np.arange(RPC, dtype=np.int64)
    for c in range(NCORES):
        rows = slice(c * RPC, (c + 1) * RPC)
        yl = y[rows]
        off = (r * V + yl).astype(np.int32).reshape(G, 128).T.copy()
        in_maps.append({"x": x[rows], "syoff": off})
    return in_maps


def _combine(per_core_osb):
    """per_core_osb: list of [128, 2G] arrays -> final scalar (np.float32).

    t/s layout: osb[p, g] is local row g*128+p.  Applies the linear
    formula after verifying the no-clip margin; exact hinge fallback
    otherwise (never taken for this problem's input distribution).
    """
    t = np.empty(B, dtype=np.float64)
    s = np.empty(B, dtype=np.float64)
    for c, o in enumerate(per_core_osb):
        o = np.asarray(o, dtype=np.float64)
        for g in range(G):
            rows = slice(c * RPC + g * 128, c * RPC + (g + 1) * 128)
            t[rows] = o[:, g]
            s[rows] = o[:, G + g]
    if 1.0 + t.min() - s.max() >= 0.0:
        out = 1.0 + t.mean() - s.mean()
    else:
        out = np.mean(np.maximum(1.0 + t[None, :] - s[:, None], 0.0))
    return np.array(out, dtype=np.float32)


def _run(x, y, k_idx=K, trace=False):
    x = np.ascontiguousarray(np.asarray(x, dtype=np.float32))
    y = np.asarray(y).astype(np.int64).reshape(B)
    assert x.shape == (B, V)

    nc = _get_nc(k_idx=k_idx)
    in_maps = _make_in_maps(x, y)
    res = run_bass_kernel_spmd(nc, in_maps, list(range(NCORES)), trace=trace)
    out = _combine([res.results[c]["osb"] for c in range(NCORES)])
    return out, res


def kernel(x, y, k):
    k = int(k)
    assert 0 <= k <= 7, f"top-8 selection supports k+1 <= 8, got k={k}"
    out, _ = _run(x, y, k_idx=k, trace=False)
    return out
